# revision 29
# baseline (speedup 1.0000x reference)
"""CDCK2 (CPC) loss kernel for 8 Trainium2 NeuronCores.

Strategy: data-parallel over batch (64 -> 8 per core). Exact BatchNorm
batch statistics via small cross-core AllReduces (one per conv layer).
conv1 stats come from a 10x10 tap-correlation matrix (sum-of-squares via
quadratic form), so conv1 output is produced only once, normalized.
conv2 output spills to DRAM; conv3/4/5 outputs stay in SBUF. GRU runs
201 unrolled steps on local batch 8. Final InfoNCE scores need pred for
the full batch -> AllGather of pred; each core computes log_softmax for
its 8 rows; host assembles nce / accuracy / h_last.
"""

import os
import numpy as np
import ml_dtypes

import concourse.bass as bass
import concourse.mybir as mybir
import concourse.tile as tile
from concourse import bacc
from concourse.bass_utils import run_bass_kernel_spmd

AF = mybir.ActivationFunctionType
ALU = mybir.AluOpType
F32 = mybir.dt.float32
F32R = mybir.dt.float32r
BF16 = mybir.dt.bfloat16

NCORES = 8
BL = 8          # local batch
B = 64          # global batch
L = 20480
J1, J2, J3 = 4096, 819, 409
TT = 12         # prediction timesteps
TS = 201        # forward seq length (t_samples + 1)
ZLEN = 213      # 201 forward + 12 encode positions
EPS = 1e-5
N1, N2, N3 = B * J1, B * J2, B * J3


def build_program(debug=False):
    nc = bacc.Bacc("TRN2", target_bir_lowering=False, debug=False,
                   num_devices=NCORES)

    # ---- external inputs (host-marshalled layouts) ----
    xt_d = nc.dram_tensor("xt", [128, BL, 32, 12], F32R, kind="ExternalInput")
    xf_d = nc.dram_tensor("xf", [10, BL, J1], F32R, kind="ExternalInput")
    w1T_d = nc.dram_tensor("w1T", [10, 512], F32R, kind="ExternalInput")
    w1P_d = nc.dram_tensor("w1P", [128, 4, 10], F32, kind="ExternalInput")
    w2T_d = nc.dram_tensor("w2T", [128, 4, 8, 512], BF16, kind="ExternalInput")
    w3T_d = nc.dram_tensor("w3T", [128, 4, 4, 512], F32R, kind="ExternalInput")
    w4T_d = nc.dram_tensor("w4T", [128, 4, 3, 512], F32R, kind="ExternalInput")
    w5T_d = nc.dram_tensor("w5T", [128, 4, 3, 512], F32R, kind="ExternalInput")
    bnp_d = nc.dram_tensor("bnp", [128, 5, 2, 4], F32, kind="ExternalInput")
    wihT_d = nc.dram_tensor("wihT", [128, 4, 768], F32R, kind="ExternalInput")
    whhT_d = nc.dram_tensor("whhT", [128, 2, 768], F32R, kind="ExternalInput")
    grub_d = nc.dram_tensor("grub", [128, 8], F32, kind="ExternalInput")
    h0_d = nc.dram_tensor("h0", [128, 2, BL], F32R, kind="ExternalInput")
    wkT_d = nc.dram_tensor("wkT", [128, 2, TT, 512], F32R, kind="ExternalInput")
    wkb_d = nc.dram_tensor("wkb", [128, 4, TT], F32, kind="ExternalInput")
    zpad_d = nc.dram_tensor("zpad", [128, 24], F32R, kind="ExternalInput")
    zpadh_d = nc.dram_tensor("zpadh", [128, 24], BF16, kind="ExternalInput")

    # ---- external outputs ----
    lsm_d = nc.dram_tensor("lsm", [BL, TT, B], F32, kind="ExternalOutput")
    hlast_d = nc.dram_tensor("hlast", [128, 2, BL], F32R, kind="ExternalOutput")
    dbg = {}
    if debug:
        for name, shape, dt_ in [("d_ab", [128, 5, 2, 4], F32),
                                 ("d_y1", [128, 4, 64], BF16),
                                 ("d_zf", [128, 4, BL, 4], F32R),
                                 ("d_xp", [128, 6, BL, 2], F32),
                                 ("d_h1", [128, 2, BL], F32R),
                                 ("d_pred", [128, 4, TT, BL], F32R),
                                 ("d_m", [10, 12], F32R),
                                 ("d_g", [128, 4, 12], F32),
                                 ("d_s2", [128, 4, 1], F32),
                                 ("d_stg2", [128, 4, 2], F32)]:
            dbg[name] = nc.dram_tensor(name, shape, dt_, kind="ExternalOutput")

    # ---- internal DRAM ----
    wu_i = nc.dram_tensor("wu_i", [128, 1], F32)
    wu_o = nc.dram_tensor("wu_o", [128, 1], F32)
    m1_i = nc.dram_tensor("m1_i", [10, 12], F32R)
    m1_o = nc.dram_tensor("m1_o", [10, 12], F32R)
    st_i = [nc.dram_tensor(f"st{l}_i", [128, 4, 2], F32) for l in (2, 3, 4, 5)]
    st_o = [nc.dram_tensor(f"st{l}_o", [128, 4, 2], F32) for l in (2, 3, 4, 5)]
    y2d = nc.dram_tensor("y2d", [BL, 128, 4, J2], F32R)
    y3d = nc.dram_tensor("y3d", [BL, 128, 4, J3], F32R)
    y4d = nc.dram_tensor("y4d", [BL, 128, 4, J3], F32R)
    pred_i = nc.dram_tensor("pred_i", [128, 4, TT, BL], F32R)
    pred_g = nc.dram_tensor("pred_g", [NCORES, 128, 4, TT, BL], F32R)

    RG = [list(range(NCORES))]

    with tile.TileContext(nc) as tc:
        pers_cm = tc.tile_pool(name="pers", bufs=1)
        pers = pers_cm.__enter__()
        dram_cm = tc.tile_pool(name="dramp", bufs=1, space="DRAM")
        dramp = dram_cm.__enter__()

        # persistent small tiles
        bnpt = pers.tile([128, 5, 2, 4], F32, name="bnpt")
        nc.sync.dma_start(bnpt[:], bnp_d[:])
        grubt = pers.tile([128, 8], F32, name="grubt")
        nc.sync.dma_start(grubt[:], grub_d[:])
        ab = pers.tile([128, 5, 2, 4], F32, name="ab")  # [l, (a|b), cc]

        # -------- warm-up collective (pays the first-collective cost early,
        # overlapped with the conv1-stats work) --------
        wu_sb = pers.tile([128, 1], F32, name="wu_sb")
        nc.any.memzero(wu_sb[:])
        nc.sync.dma_start(wu_i[:], wu_sb[:])
        nc.gpsimd.collective_compute("AllReduce", ALU.add, replica_groups=RG,
                                     ins=[wu_i[:]], outs=[wu_o[:]])

        def bn_tail(pool, ps_small, l, sumy, sumy2, n_elems):
            """sumy/sumy2: [128, 4] APs of global sums -> ab[:, l] coeffs."""
            inv_n = 1.0 / float(n_elems)
            mean = pool.tile([128, 4], F32, name=f"mean{l}", tag="bn_mean")
            nc.vector.tensor_scalar_mul(mean[:], sumy, inv_n)
            ex2 = pool.tile([128, 4], F32, name=f"ex2{l}", tag="bn_ex2")
            nc.vector.tensor_scalar_mul(ex2[:], sumy2, inv_n)
            m2 = pool.tile([128, 4], F32, name=f"m2{l}", tag="bn_m2")
            nc.vector.tensor_mul(out=m2[:], in0=mean[:], in1=mean[:])
            var = pool.tile([128, 4], F32, name=f"var{l}", tag="bn_var")
            nc.vector.tensor_tensor(out=var[:], in0=ex2[:], in1=m2[:],
                                    op=ALU.subtract)
            vpe = pool.tile([128, 4], F32, name=f"vpe{l}", tag="bn_vpe")
            nc.vector.tensor_scalar_add(out=vpe[:], in0=var[:], scalar1=EPS)
            lnv = pool.tile([128, 4], F32, name=f"lnv{l}", tag="bn_lnv")
            nc.scalar.activation(out=lnv[:], in_=vpe[:], func=AF.Ln)
            inv = pool.tile([128, 4], F32, name=f"inv{l}", tag="bn_inv")
            nc.scalar.activation(out=inv[:], in_=lnv[:], func=AF.Exp, scale=-0.5)
            # a = gamma * inv ; b = beta - mean * a
            nc.vector.tensor_mul(out=ab[:, l, 0, :], in0=inv[:],
                                 in1=bnpt[:, l, 0, :])
            ma = pool.tile([128, 4], F32, name=f"ma{l}", tag="bn_ma")
            nc.vector.tensor_mul(out=ma[:], in0=mean[:], in1=ab[:, l, 0, :])
            nc.vector.tensor_tensor(out=ab[:, l, 1, :], in0=bnpt[:, l, 1, :],
                                    in1=ma[:], op=ALU.subtract)

        # ================= phase 1: conv1 stats -> AR -> conv1+conv2 ========
        ph1_cm = tc.tile_pool(name="ph1", bufs=1)
        ph1 = ph1_cm.__enter__()

        xt = ph1.tile([128, BL, 32, 12], F32R, name="xt")
        nc.sync.dma_start(xt[:], xt_d[:])
        w1T = ph1.tile([10, 512], F32R, name="w1T")
        nc.sync.dma_start(w1T[:], w1T_d[:])
        w1P = ph1.tile([128, 4, 10], F32, name="w1P")
        nc.sync.dma_start(w1P[:], w1P_d[:])
        w2T = ph1.tile([128, 4, 8, 512], BF16, name="w2T")
        nc.sync.dma_start(w2T[:], w2T_d[:])

        # conv1 stats: M = sum_j taps_j taps_j^T (+ sum column via ones)
        st1ps_cm = tc.tile_pool(name="st1ps", bufs=1, space="PSUM")
        st1ps = st1ps_cm.__enter__()
        mm_ps = st1ps.tile([10, 12], F32, name="mm_ps")
        n_acc = BL * 32
        idx = 0
        for b in range(BL):
            for i in range(32):
                nc.tensor.matmul(mm_ps[:], lhsT=xt[:, b, i, 0:10],
                                 rhs=xt[:, b, i, 0:12],
                                 start=(idx == 0), stop=(idx == n_acc - 1))
                idx += 1
        m_sb = ph1.tile([10, 12], F32R, name="m_sb")
        nc.scalar.copy(out=m_sb[:], in_=mm_ps[:])
        nc.sync.dma_start(m1_i[:], m_sb[:])
        nc.gpsimd.collective_compute("AllReduce", ALU.add, replica_groups=RG,
                                     ins=[m1_i[:]], outs=[m1_o[:]])
        m_g = ph1.tile([10, 12], F32R, name="m_g")
        nc.sync.dma_start(m_g[:], m1_o[:])

        # BN1 coeffs: G = W1^T M ; sumy2 = sum_k G[:, k] * w1 ; sumy = G[:, 10]
        g1 = ph1.tile([128, 4, 12], F32, name="g1")
        for cc in range(4):
            g_ps = st1ps.tile([128, 12], F32, name=f"g_ps{cc}", tag="g_ps")
            nc.tensor.matmul(g_ps[:], lhsT=w1T[:, cc * 128:(cc + 1) * 128],
                             rhs=m_g[:], start=True, stop=True)
            nc.scalar.copy(out=g1[:, cc, :], in_=g_ps[:])
        gm = ph1.tile([128, 4, 10], F32, name="gm")
        nc.vector.tensor_mul(out=gm[:], in0=g1[:, :, 0:10], in1=w1P[:])
        s2_1 = ph1.tile([128, 4, 1], F32, name="s2_1")
        nc.vector.reduce_sum(out=s2_1[:], in_=gm[:], axis=mybir.AxisListType.X)
        bn_tail(ph1, None, 0, g1[:, :, 10], s2_1[:, :, 0], N1)
        st1ps_cm.__exit__(None, None, None)
        ph1ps_cm = tc.tile_pool(name="ph1ps", bufs=2, space="PSUM")
        ph1ps = ph1ps_cm.__enter__()
        if debug:
            nc.sync.dma_start(dbg["d_m"][:], m_g[:])
            nc.sync.dma_start(dbg["d_g"][:], g1[:])
            nc.sync.dma_start(dbg["d_s2"][:], s2_1[:])

        # conv1 (BN-folded, relu) + conv2 (+ stats), batch by batch
        y1 = ph1.tile([128, 4, J1 + 8], BF16, name="y1")
        sum2 = ph1.tile([128, 4, 16], F32, name="sum2")
        sq2 = ph1.tile([128, 4, 16], F32, name="sq2")
        for b in range(BL):
            xfb = ph1.tile([10, J1], F32R, name=f"xfb{b}", tag="xfb", bufs=2)
            nc.sync.dma_start(xfb[:], xf_d[:, b, :])
            nc.sync.dma_start(y1[:, :, 0:2], zpadh_d[:, 0:8].rearrange("p (a c) -> p a c", a=4))
            nc.sync.dma_start(y1[:, :, J1 + 2:J1 + 8],
                              zpadh_d[:].rearrange("p (a c) -> p a c", a=4))
            for cc in range(4):
                for jc in range(4):   # 1024-wide units: 2 matmuls each
                    p1 = ph1ps.tile([128, 1024], F32, name=f"p1_{b}_{cc}_{jc}",
                                    tag="p1", bufs=2)
                    for half in range(2):
                        j0 = jc * 1024 + half * 512
                        nc.tensor.matmul(p1[:, half * 512:(half + 1) * 512],
                                         lhsT=w1T[:, cc * 128:(cc + 1) * 128],
                                         rhs=xfb[:, j0:j0 + 512],
                                         start=True, stop=True)
                    dst = y1[:, cc, 2 + jc * 1024: 2 + (jc + 1) * 1024]
                    if jc % 2 == 0:
                        nc.scalar.activation(
                            out=dst, in_=p1[:], func=AF.Relu,
                            scale=ab[:, 0, 0, cc:cc + 1],
                            bias=ab[:, 0, 1, cc:cc + 1])
                    else:
                        t1 = ph1.tile([128, 1024], F32, name=f"t1_{b}_{cc}_{jc}",
                                      tag="t1", bufs=2)
                        nc.vector.tensor_scalar(
                            out=t1[:], in0=p1[:],
                            scalar1=ab[:, 0, 0, cc:cc + 1],
                            scalar2=ab[:, 0, 1, cc:cc + 1],
                            op0=ALU.mult, op1=ALU.add)
                        nc.vector.tensor_scalar_max(out=dst, in0=t1[:],
                                                    scalar1=0.0)
            if debug and b == 0:
                nc.sync.dma_start(dbg["d_y1"][:], y1[:, :, 2:66])
            for coc in range(4):
                for jci, (j0, jw) in enumerate([(0, 512), (512, 308)]):
                    p2 = ph1ps.tile([128, 512], F32, name=f"p2_{b}_{coc}_{jci}",
                                    tag="p2")
                    ki = 0
                    for cc in range(4):
                        for k in range(8):
                            r0 = k + 5 * j0
                            nc.tensor.matmul(
                                p2[:, :jw],
                                lhsT=w2T[:, cc, k, coc * 128:(coc + 1) * 128],
                                rhs=y1[:, cc, r0: r0 + 5 * (jw - 1) + 1: 5],
                                start=(ki == 0), stop=(ki == 31))
                            ki += 1
                    jv = min(jw, J2 - j0)
                    slot = b * 2 + jci
                    nc.vector.reduce_sum(out=sum2[:, coc, slot:slot + 1],
                                         in_=p2[:, :jv],
                                         axis=mybir.AxisListType.X)
                    ysq = ph1.tile([128, 512], F32, name=f"ysq_{b}_{coc}_{jci}",
                                   tag="ysq", bufs=2)
                    nc.scalar.activation(out=ysq[:, :jv], in_=p2[:, :jv],
                                         func=AF.Square,
                                         accum_out=sq2[:, coc, slot:slot + 1])
                    yst = ph1.tile([128, 512], F32R, name=f"yst_{b}_{coc}_{jci}",
                                   tag="yst", bufs=3)
                    nc.scalar.copy(out=yst[:, :jv], in_=p2[:, :jv])
                    nc.sync.dma_start(y2d[b, :, coc, j0:j0 + jv], yst[:, :jv])

        # conv2 BN stats AR
        stp2 = ph1.tile([128, 4, 2], F32, name="stp2")
        nc.vector.reduce_sum(out=stp2[:, :, 0:1], in_=sum2[:],
                             axis=mybir.AxisListType.X)
        nc.vector.reduce_sum(out=stp2[:, :, 1:2], in_=sq2[:],
                             axis=mybir.AxisListType.X)
        nc.sync.dma_start(st_i[0][:], stp2[:])
        nc.gpsimd.collective_compute("AllReduce", ALU.add, replica_groups=RG,
                                     ins=[st_i[0][:]], outs=[st_o[0][:]])
        stg2 = ph1.tile([128, 4, 2], F32, name="stg2")
        nc.sync.dma_start(stg2[:], st_o[0][:])
        bn_tail(ph1, None, 1, stg2[:, :, 0], stg2[:, :, 1], N2)
        if debug:
            nc.sync.dma_start(dbg["d_stg2"][:], stg2[:])
            nc.sync.dma_start(dbg["d_ab"][:], ab[:])

        ph1ps_cm.__exit__(None, None, None)
        ph1_cm.__exit__(None, None, None)

        # ============ conv3/4/5: DRAM -> DRAM (conv5 -> SBUF) ==============
        late_cm = tc.tile_pool(name="late", bufs=1)
        late = late_cm.__enter__()
        zf = late.tile([128, 4, BL, ZLEN], F32R, name="zf")
        y5p_cm = tc.tile_pool(name="y5p", bufs=1)
        y5p = y5p_cm.__enter__()
        y5 = y5p.tile([128, 4, BL, J3], F32R, name="y5")

        def conv_mid(lidx, wT_d, K, stride, src_d, dst_d, dst_sb, n_elems, wK):
            """lidx: bn layer index of INPUT normalization (src is pre-BN).
            src_d: DRAM [BL, 128, 4, Lin]. dst_d: DRAM or None; dst_sb: tile
            [128, 4, BL, J3] or None. Computes this layer's stats -> AR ->
            bn coeffs for layer lidx+1.
            """
            pool_cm = tc.tile_pool(name=f"cv{lidx}", bufs=1)
            pool = pool_cm.__enter__()
            ps_cm = tc.tile_pool(name=f"cv{lidx}ps", bufs=2, space="PSUM")
            ps = ps_cm.__enter__()
            Lin = J2 if lidx == 1 else J3
            JP = J3 + 1          # even matmul width; last col is garbage
            maxi = stride * (JP - 1) + (K - 1) - 1
            rpad = max(2, maxi - (Lin - 1))
            W = 1 + Lin + rpad
            wT = pool.tile([128, 4, wK, 512], F32R, name=f"wT{lidx}")
            nc.sync.dma_start(wT[:], wT_d[:])
            sums = pool.tile([128, 4, BL], F32, name=f"sums{lidx}")
            sqs = pool.tile([128, 4, BL], F32, name=f"sqs{lidx}")
            for b in range(BL):
                raw = pool.tile([128, 4, Lin], F32R, name=f"raw{lidx}_{b}",
                                tag=f"raw{lidx}", bufs=2)
                nc.sync.dma_start(raw[:], src_d[b])
                xb = pool.tile([128, 4, W], F32R, name=f"xb{lidx}_{b}",
                               tag=f"xb{lidx}", bufs=2)
                nc.sync.dma_start(xb[:, :, 0:1],
                                  zpad_d[:, 0:4].rearrange("p (a c) -> p a c", a=4))
                nc.sync.dma_start(xb[:, :, W - rpad:W],
                                  zpad_d[:, 0:4 * rpad].rearrange(
                                      "p (a c) -> p a c", a=4))
                for cc in range(4):
                    nc.scalar.activation(out=xb[:, cc, 1:1 + Lin],
                                         in_=raw[:, cc, :], func=AF.Relu,
                                         scale=ab[:, lidx, 0, cc:cc + 1],
                                         bias=ab[:, lidx, 1, cc:cc + 1])
                for coc in range(4):
                    pm = ps.tile([128, JP], F32, name=f"pm{lidx}_{b}_{coc}",
                                 tag=f"pm{lidx}")
                    ki = 0
                    nk = 4 * K
                    for cc in range(4):
                        for k in range(K):
                            nc.tensor.matmul(
                                pm[:],
                                lhsT=wT[:, cc, k, coc * 128:(coc + 1) * 128],
                                rhs=xb[:, cc, k: k + stride * (JP - 1) + 1: stride],
                                start=(ki == 0), stop=(ki == nk - 1))
                            ki += 1
                    nc.vector.reduce_sum(out=sums[:, coc, b:b + 1],
                                         in_=pm[:, 0:J3], axis=mybir.AxisListType.X)
                    ysq = pool.tile([128, J3], F32, name=f"ysqm{lidx}_{b}_{coc}",
                                    tag=f"ysqm{lidx}", bufs=2)
                    nc.scalar.activation(out=ysq[:], in_=pm[:, 0:J3], func=AF.Square,
                                         accum_out=sqs[:, coc, b:b + 1])
                    if dst_sb is not None:
                        nc.scalar.copy(out=dst_sb[:, coc, b, :], in_=pm[:, 0:J3])
                    else:
                        yst = pool.tile([128, J3], F32R, name=f"yst{lidx}_{b}_{coc}",
                                        tag=f"yst{lidx}", bufs=3)
                        nc.scalar.copy(out=yst[:], in_=pm[:, 0:J3])
                        nc.sync.dma_start(dst_d[b, :, coc, :], yst[:])
            sti, sto = st_i[lidx], st_o[lidx]
            stp = pool.tile([128, 4, 2], F32, name=f"stp{lidx}")
            nc.vector.reduce_sum(out=stp[:, :, 0:1], in_=sums[:],
                                 axis=mybir.AxisListType.X)
            nc.vector.reduce_sum(out=stp[:, :, 1:2], in_=sqs[:],
                                 axis=mybir.AxisListType.X)
            nc.sync.dma_start(sti[:], stp[:])
            nc.gpsimd.collective_compute("AllReduce", ALU.add,
                                         replica_groups=RG,
                                         ins=[sti[:]], outs=[sto[:]])
            stg = pool.tile([128, 4, 2], F32, name=f"stg{lidx}")
            nc.sync.dma_start(stg[:], sto[:])
            bn_tail(pool, ps, lidx + 1, stg[:, :, 0], stg[:, :, 1], n_elems)
            ps_cm.__exit__(None, None, None)
            pool_cm.__exit__(None, None, None)

        # conv3: reads y2 (BN1-normalized on load), K=4 s=2 -> y3d, BN3 coeffs
        conv_mid(1, w3T_d, 4, 2, y2d, y3d, None, N3, 4)
        # conv4: reads y3 (BN3), K=3 s=1 -> y4d, BN4 coeffs
        conv_mid(2, w4T_d, 3, 1, y3d, y4d, None, N3, 3)
        # conv5: reads y4 (BN4), K=3 s=1 -> y5 (SBUF), BN5 coeffs
        conv_mid(3, w5T_d, 3, 1, y4d, None, y5, N3, 3)

        # ================= z = BN5+relu of y5[:213] =========================
        for cc in range(4):
            for b in range(BL):
                nc.scalar.activation(out=zf[:, cc, b, :], in_=y5[:, cc, b, 0:ZLEN],
                                     func=AF.Relu, scale=ab[:, 4, 0, cc:cc + 1],
                                     bias=ab[:, 4, 1, cc:cc + 1])
        if debug:
            nc.sync.dma_start(dbg["d_zf"][:], zf[:, :, :, 0:4])
        y5p_cm.__exit__(None, None, None)

        # ================= GRU ==============================================
        gru_cm = tc.tile_pool(name="gru", bufs=1)
        gru = gru_cm.__enter__()
        grups_cm = tc.tile_pool(name="grups", bufs=2, space="PSUM")
        grups = grups_cm.__enter__()

        wih = gru.tile([128, 4, 768], F32R, name="wih")
        nc.sync.dma_start(wih[:], wihT_d[:])
        whh = gru.tile([128, 2, 768], F32R, name="whh")
        nc.sync.dma_start(whh[:], whhT_d[:])

        # x_proj[g, b, t] = (z @ wih^T)[g] + bias(g)   (bias = bih + bhh_rz)
        # inner width 202 (f32r matmuls need even moving dims); t=201 unused
        xproj = gru.tile([128, 6, BL, TS + 1], F32, name="xproj")
        for gc in range(6):
            for cb in range(4):
                xp_ps = grups.tile([128, 2, TS + 1], F32, name=f"xp_{gc}_{cb}",
                                   tag="xp_ps")
                for cc in range(4):
                    nc.tensor.matmul(xp_ps[:],
                                     lhsT=wih[:, cc, gc * 128:(gc + 1) * 128],
                                     rhs=zf[:, cc, 2 * cb:2 * cb + 2, 0:TS + 1],
                                     start=(cc == 0), stop=(cc == 3))
                nc.scalar.activation(out=xproj[:, gc, 2 * cb:2 * cb + 2, :],
                                     in_=xp_ps[:], func=AF.Identity,
                                     bias=grubt[:, gc:gc + 1])
        if debug:
            nc.sync.dma_start(dbg["d_xp"][:], xproj[:, :, :, 0:2])

        hpool_cm = tc.tile_pool(name="hpool", bufs=3)
        hpool = hpool_cm.__enter__()
        gw_cm = tc.tile_pool(name="gw", bufs=3)
        gw = gw_cm.__enter__()

        h = hpool.tile([128, 2, BL], F32R, name="h_init", tag="h")
        nc.sync.dma_start(h[:], h0_d[:])

        for t in range(TS):
            # r-gate matmuls first: the sigmoid->tanh chain is the critical
            # path; z/n matmuls overlap with it
            hp_r = grups.tile([128, 2, BL], F32, name=f"hpr{t}", tag="hp_r", bufs=1)
            for gc in range(2):
                for hc in range(2):
                    nc.tensor.matmul(hp_r[:, gc, :],
                                     lhsT=whh[:, hc, gc * 128:(gc + 1) * 128],
                                     rhs=h[:, hc, :],
                                     start=(hc == 0), stop=(hc == 1))
            hp_n = grups.tile([128, 2, BL], F32, name=f"hpn{t}", tag="hp_n", bufs=1)
            for gc in range(2):
                for hc in range(2):
                    nc.tensor.matmul(hp_n[:, gc, :],
                                     lhsT=whh[:, hc, (4 + gc) * 128:(5 + gc) * 128],
                                     rhs=h[:, hc, :],
                                     start=(hc == 0), stop=(hc == 1))
            hp_z = grups.tile([128, 2, BL], F32, name=f"hpz{t}", tag="hp_z", bufs=1)
            for gc in range(2):
                for hc in range(2):
                    nc.tensor.matmul(hp_z[:, gc, :],
                                     lhsT=whh[:, hc, (2 + gc) * 128:(3 + gc) * 128],
                                     rhs=h[:, hc, :],
                                     start=(hc == 0), stop=(hc == 1))
            sr = gw.tile([128, 2, BL], F32, name=f"sr{t}", tag="sr")
            nc.vector.tensor_tensor(out=sr[:], in0=hp_r[:],
                                    in1=xproj[:, 0:2, :, t], op=ALU.add)
            r = gw.tile([128, 2, BL], F32, name=f"r{t}", tag="r")
            nc.scalar.activation(out=r[:], in_=sr[:], func=AF.Sigmoid)
            hnb = gw.tile([128, 2, BL], F32, name=f"hnb{t}", tag="hnb")
            nc.vector.tensor_tensor(out=hnb[:], in0=hp_n[:],
                                    in1=grubt[:, 6:8, None].to_broadcast(
                                        [128, 2, BL]), op=ALU.add)
            rhn = gw.tile([128, 2, BL], F32, name=f"rhn{t}", tag="rhn")
            nc.vector.tensor_mul(out=rhn[:], in0=r[:], in1=hnb[:])
            nin = gw.tile([128, 2, BL], F32, name=f"nin{t}", tag="nin")
            nc.vector.tensor_tensor(out=nin[:], in0=rhn[:],
                                    in1=xproj[:, 4:6, :, t], op=ALU.add)
            n = gw.tile([128, 2, BL], F32, name=f"n{t}", tag="n")
            nc.scalar.activation(out=n[:], in_=nin[:], func=AF.Tanh)
            # z branch (overlaps the n branch)
            sz = gw.tile([128, 2, BL], F32, name=f"sz{t}", tag="sz")
            nc.vector.tensor_tensor(out=sz[:], in0=hp_z[:],
                                    in1=xproj[:, 2:4, :, t], op=ALU.add)
            z = gw.tile([128, 2, BL], F32, name=f"z{t}", tag="z")
            nc.scalar.activation(out=z[:], in_=sz[:], func=AF.Sigmoid)
            zh = gw.tile([128, 2, BL], F32, name=f"zh{t}", tag="zh")
            nc.vector.tensor_mul(out=zh[:], in0=z[:], in1=h[:])
            omz = gw.tile([128, 2, BL], F32, name=f"omz{t}", tag="omz")
            nc.vector.tensor_scalar(out=omz[:], in0=z[:],
                                    scalar1=-1.0, scalar2=1.0,
                                    op0=ALU.mult, op1=ALU.add)
            on = gw.tile([128, 2, BL], F32, name=f"on{t}", tag="on")
            nc.vector.tensor_mul(out=on[:], in0=omz[:], in1=n[:])
            h_new = hpool.tile([128, 2, BL], F32R, name=f"h{t + 1}", tag="h")
            nc.vector.tensor_tensor(out=h_new[:], in0=on[:], in1=zh[:],
                                    op=ALU.add)
            h = h_new
            if debug and t == 0:
                nc.sync.dma_start(dbg["d_h1"][:], h[:])

        nc.sync.dma_start(hlast_d[:], h[:])

        # ================= pred + AllGather ================================
        wkTt = gru.tile([128, 2, TT, 512], F32R, name="wkTt")
        nc.sync.dma_start(wkTt[:], wkT_d[:])
        wkbt = gru.tile([128, 4, TT], F32, name="wkbt")
        nc.sync.dma_start(wkbt[:], wkb_d[:])
        predloc = gru.tile([128, 4, TT, BL], F32R, name="predloc")
        for tt in range(TT):
            for kc in range(4):
                pp = grups.tile([128, BL], F32, name=f"pp{tt}_{kc}", tag="pp", bufs=1)
                for hc in range(2):
                    nc.tensor.matmul(pp[:],
                                     lhsT=wkTt[:, hc, tt, kc * 128:(kc + 1) * 128],
                                     rhs=h[:, hc, :],
                                     start=(hc == 0), stop=(hc == 1))
                nc.scalar.activation(out=predloc[:, kc, tt, :], in_=pp[:],
                                     func=AF.Identity,
                                     bias=wkbt[:, kc, tt:tt + 1])
        if debug:
            nc.sync.dma_start(dbg["d_pred"][:], predloc[:])
        nc.sync.dma_start(pred_i[:], predloc[:])
        nc.gpsimd.collective_compute("AllGather", ALU.bypass, replica_groups=RG,
                                     ins=[pred_i[:]], outs=[pred_g[:]])
        predsb = gru.tile([128, 4, NCORES, TT, BL], F32R, name="predsb")
        nc.sync.dma_start(predsb[:],
                          pred_g.ap().rearrange("c p k t b -> p k c t b"))

        # ================= scores + log_softmax ============================
        sc_sb = gru.tile([BL, TT, B], F32, name="sc_sb")
        for tt in range(TT):
            sc_ps = grups.tile([BL, B], F32, name=f"scps{tt}", tag="sc_ps", bufs=1)
            for kc in range(4):
                nc.tensor.matmul(sc_ps[:],
                                 lhsT=zf[:, kc, :, TS + tt],
                                 rhs=predsb[:, kc, :, tt, :],
                                 start=(kc == 0), stop=(kc == 3))
            nc.scalar.copy(out=sc_sb[:, tt, :], in_=sc_ps[:])
        nmx = gru.tile([BL, TT, 1], F32, name="nmx")
        nc.vector.tensor_reduce(out=nmx[:], in_=sc_sb[:], op=ALU.max,
                                axis=mybir.AxisListType.X, negate=True)
        sh = gru.tile([BL, TT, B], F32, name="sh")
        nc.vector.tensor_tensor(out=sh[:], in0=sc_sb[:],
                                in1=nmx[:].to_broadcast([BL, TT, B]), op=ALU.add)
        ex = gru.tile([BL, TT, B], F32, name="ex")
        nc.scalar.activation(out=ex[:], in_=sh[:], func=AF.Exp)
        se = gru.tile([BL, TT, 1], F32, name="se")
        nc.vector.reduce_sum(out=se[:], in_=ex[:], axis=mybir.AxisListType.X)
        lse = gru.tile([BL, TT, 1], F32, name="lse")
        nc.scalar.activation(out=lse[:], in_=se[:], func=AF.Ln)
        lsmt = gru.tile([BL, TT, B], F32, name="lsmt")
        nc.vector.tensor_tensor(out=lsmt[:], in0=sh[:],
                                in1=lse[:].to_broadcast([BL, TT, B]),
                                op=ALU.subtract)
        nc.sync.dma_start(lsm_d[:], lsmt[:])

        gw_cm.__exit__(None, None, None)
        hpool_cm.__exit__(None, None, None)
        grups_cm.__exit__(None, None, None)
        gru_cm.__exit__(None, None, None)
        late_cm.__exit__(None, None, None)
        dram_cm.__exit__(None, None, None)
        pers_cm.__exit__(None, None, None)

    nc.compile()
    return nc


# ---------------------------------------------------------------------------
# host-side marshalling
# ---------------------------------------------------------------------------

def _prep_inputs(x, hidden, w1, w2, w3, w4, w5, bn_gamma, bn_beta,
                 gru_wih, gru_whh, gru_bih, gru_bhh, wk_w, wk_b):
    f32 = np.float32
    x = np.asarray(x, f32).reshape(B, L)
    # padded signal: xp[m] = x[m - 3]
    xp = np.zeros((B, L + 16), f32)
    xp[:, 3:3 + L] = x

    # xf[k, b, j] = xp[b, 5j + k]
    sv = np.lib.stride_tricks.sliding_window_view(xp, 10, axis=1)  # [B, L+7, 10]
    xf_all = np.ascontiguousarray(
        sv[:, 0:5 * J1:5, :].transpose(2, 0, 1))  # [10, B, J1]

    # xt[p, b, i, c] = xp[b, 5*(128 i + p) + c], c<10 ; c==10 -> 1
    taps = sv[:, 0:5 * J1:5, :]                      # [B, J1, 10]
    xt_all = np.zeros((B, J1, 12), f32)
    xt_all[:, :, 0:10] = taps
    xt_all[:, :, 10] = 1.0
    xt_all = np.ascontiguousarray(
        xt_all.reshape(B, 32, 128, 12).transpose(2, 0, 1, 3))  # [128, B, 32, 12]

    w1 = np.asarray(w1, f32).reshape(512, 10)
    w1T = np.ascontiguousarray(w1.T)                              # [10, 512]
    w1P = np.ascontiguousarray(w1.reshape(4, 128, 10).transpose(1, 0, 2))

    def conv_wT(w, K):
        w = np.asarray(w, f32)          # [512, 512, K]
        arr = w.transpose(1, 2, 0)      # [cin, k, co]
        return np.ascontiguousarray(
            arr.reshape(4, 128, K, 512).transpose(1, 0, 2, 3))

    w2T = conv_wT(w2, 8).astype(ml_dtypes.bfloat16)
    w3T = conv_wT(w3, 4)
    w4T = conv_wT(w4, 3)
    w5T = conv_wT(w5, 3)

    bnp = np.stack([np.asarray(bn_gamma, f32), np.asarray(bn_beta, f32)],
                   axis=1)                                  # [5, 2, 512]
    bnp = np.ascontiguousarray(
        bnp.reshape(5, 2, 4, 128).transpose(3, 0, 1, 2))    # [128, 5, 2, 4]

    wihT = np.ascontiguousarray(
        np.asarray(gru_wih, f32).T.reshape(4, 128, 768).transpose(1, 0, 2))
    whhT = np.ascontiguousarray(
        np.asarray(gru_whh, f32).T.reshape(2, 128, 768).transpose(1, 0, 2))
    xb = np.asarray(gru_bih, f32).copy()
    xb[0:512] += np.asarray(gru_bhh, f32)[0:512]
    grub = np.zeros((128, 8), f32)
    grub[:, 0:6] = xb.reshape(6, 128).T
    grub[:, 6:8] = np.asarray(gru_bhh, f32)[512:768].reshape(2, 128).T

    wkT = np.ascontiguousarray(
        np.asarray(wk_w, f32).transpose(2, 0, 1)            # [256, T, 512]
        .reshape(2, 128, TT, 512).transpose(1, 0, 2, 3))    # [128, 2, T, 512]
    wkb = np.ascontiguousarray(
        np.asarray(wk_b, f32).T.reshape(4, 128, TT).transpose(1, 0, 2))

    hid = np.asarray(hidden, f32)[0]                        # [B, 256]

    shared = dict(w1T=w1T, w1P=w1P, w2T=w2T, w3T=w3T, w4T=w4T, w5T=w5T,
                  bnp=bnp, wihT=wihT, whhT=whhT, grub=grub, wkT=wkT, wkb=wkb,
                  zpad=np.zeros((128, 24), f32),
                  zpadh=np.zeros((128, 24), ml_dtypes.bfloat16))
    in_maps = []
    for c in range(NCORES):
        bs = slice(c * BL, (c + 1) * BL)
        h0 = np.ascontiguousarray(
            hid[bs].T.reshape(2, 128, BL).transpose(1, 0, 2))  # [128, 2, BL]
        m = dict(shared)
        m["xt"] = np.ascontiguousarray(xt_all[:, bs])
        m["xf"] = np.ascontiguousarray(xf_all[:, bs])
        m["h0"] = h0
        in_maps.append(m)
    return in_maps


def _assemble(results):
    lsm = np.concatenate([results[c]["lsm"] for c in range(NCORES)], axis=0)
    # lsm[b, t, c] = log_softmax(total)[t, b, c]
    bidx = np.arange(B)
    nce = np.float32(lsm[bidx, :, bidx].sum() / (-1.0 * B * TT))
    sm_last = np.exp(lsm[:, TT - 1, :])          # softmax(total[-1], axis=-1)
    correct = (np.argmax(sm_last, axis=0) == bidx).sum()
    accuracy = np.float32(correct / B)
    h = np.concatenate(
        [results[c]["hlast"].transpose(2, 1, 0).reshape(BL, 256)
         for c in range(NCORES)], axis=0)        # [B, 256]
    return accuracy, nce, h[None].astype(np.float32)


_CACHED_NC = None


def kernel(**inputs):
    global _CACHED_NC
    if _CACHED_NC is None:
        _CACHED_NC = build_program(debug=False)
    in_maps = _prep_inputs(
        inputs["x"], inputs["hidden"], inputs["w1"], inputs["w2"], inputs["w3"],
        inputs["w4"], inputs["w5"], inputs["bn_gamma"], inputs["bn_beta"],
        inputs["gru_wih"], inputs["gru_whh"], inputs["gru_bih"],
        inputs["gru_bhh"], inputs["wk_w"], inputs["wk_b"])
    res = run_bass_kernel_spmd(_CACHED_NC, in_maps, list(range(NCORES)))
    return _assemble(res.results)
